# revision 97
# baseline (speedup 1.0000x reference)
"""Causal dense attention (key=value) on 8 TRN2 NeuronCores.

Reference semantics (B=4, T=2048, D=1024, fp32):
    scores  = Q @ V^T                      [B, T, T]
    scores -= 1e9 * (~tril)                causal mask
    W       = softmax(scores, axis=-1)
    out     = W @ V                        [B, T, D]

Sharding: 2 cores per batch. Each batch's 16 causal q-tiles (128 rows
each, kv extent 128*(t+1)) are split odd/even so both cores get the
same padded kv-extent schedule EXT = [256, 512, ..., 2048] (ascending),
making the Bass program identical across all 8 cores (pure SPMD).
Padding columns are killed by the additive causal mask, which is the
same [128, 256] pattern for every slot of a given core.

Host stages per core: Q^T (d-major, columns grouped by processing
order) and V^T (d-major) in fp16 (full TensorE rate at half the HBM
bytes of f32r; scores keep ~10 mantissa bits), V split into fp8e4
hi/lo halves (V = hi + lo at ~bf16 effective precision), and one
additive causal mask tile.

Device pipeline per slot (software-pipelined, lag 2, emitted as
tr(j-2) -> mm1(j) -> mm2(j-2) so the PE never waits on the softmax
handoff):
  mm1  S = Q^T.T @ V^T into PSUM kilo-windows (fp16); DVE adds the
       causal mask to the last 256 columns;
  stats row-max (DVE reduce, negated) -> exp with fused bias to fp8e4
       W in SBUF (ScalarE); DVE row-sum over the quantized W so the
       softmax normalizer matches the fp8 weights exactly;
  mm2  PE-transpose of W blocks (fp8, stride-2 PSUM out), repack on
       ScalarE, then DoubleRow fp8 matmuls W^T.T @ V_hi + W^T.T @ V_lo
       (two k-tiles per instruction, 2x rate) into PSUM, and a ScalarE
       copy fused with the 1/rowsum scale emitting bf16, then DMA out
       from the ACT queue (keeps the SP queue free for inputs).
Input DMAs are coalesced (one strided descriptor-set per wave) and
ordered by consumption; the DMA engine runs gapless until all 10.1
MiB of input have landed. The last three backs are hand-interleaved:
the big mm2 runs last in chunk-major order so its out-copy/DMA chains
pipeline under its own accumulation.
"""

import numpy as np

import concourse.bass as bass
import concourse.mybir as mybir
from concourse import bacc, tile
from concourse.bass_utils import run_bass_kernel_spmd
from concourse.masks import make_identity

import ml_dtypes

B, T, D = 4, 2048, 1024
NCORES = 8
NSLOT = 8
EXT = [256 * (j + 1) for j in range(NSLOT)]  # kv extent per slot
# processing order: smallest slot first (fast start); the two
# cheapest remaining backs (slots 2 and 0) drain the pipeline tail
ORDER = [1, 3, 4, 5, 6, 7, 2, 0]
# qt DRAM column-blocks are laid out in ORDER; waves below are
# (start index into ORDER, slot count) — multi-slot waves keep the
# DRAM descriptor runs >= 512B
QT_WAVES = [(0, 2), (2, 2), (4, 2), (6, 2)]
NEG_INF = 1e9
PE_WARMUP = 24  # dependency-free PE transposes at program start

F32 = mybir.dt.float32
F16 = mybir.dt.float16
BF16 = mybir.dt.bfloat16
FP8 = mybir.dt.float8e4
E4 = ml_dtypes.float8_e4m3


def _tiles_for_core(c):
    """q-tile index (within the batch) for each slot, for core c."""
    if c < 4:
        return [2 * j + 1 for j in range(NSLOT)]  # extents exactly EXT
    return [2 * j for j in range(NSLOT)]  # extents EXT - 128 (padded)


def _build_program():
    nc = bacc.Bacc("TRN2", target_bir_lowering=False)

    qt_d = nc.dram_tensor("qt", [D, NSLOT * 128], F16, kind="ExternalInput")
    vt_d = nc.dram_tensor("vt", [D, T], F16, kind="ExternalInput")
    vhl_d = nc.dram_tensor("vhl", [2, T, D], FP8, kind="ExternalInput")
    mask_d = nc.dram_tensor("mask", [128, 256], F32, kind="ExternalInput")
    o_d = nc.dram_tensor("o", [NSLOT * 128, D], BF16, kind="ExternalOutput")

    with tile.TileContext(nc) as tc:
        with (
            tc.tile_pool(name="const", bufs=1) as constp,
            tc.tile_pool(name="qt", bufs=1) as qtp,
            tc.tile_pool(name="vt", bufs=1) as vtp,
            tc.tile_pool(name="vn", bufs=1) as vnp,
            tc.tile_pool(name="w", bufs=3) as wp,
            tc.tile_pool(name="wt", bufs=8) as wtp,
            tc.tile_pool(name="osb", bufs=4) as op,
            tc.tile_pool(name="stats", bufs=24) as statp,
            tc.tile_pool(name="ps_s", bufs=4, space="PSUM") as ps_s,
            tc.tile_pool(name="ps_t", bufs=2, space="PSUM") as ps_t,
            tc.tile_pool(name="ps_o", bufs=2, space="PSUM") as ps_o,
        ):
            ident = constp.tile([128, 128], FP8, tag="ident")
            make_identity(nc, ident[:])

            # ACT exp-table warm-up: load exp_and_others during initial DMAs
            warm = statp.tile([128, 1], F32, tag="warm")
            nc.gpsimd.memset(warm[:], 0.0)
            nc.scalar.activation(warm[:], warm[:], mybir.ActivationFunctionType.Exp)

            # PE p-state warm-up: dependency-free transposes keep the PE
            # busy while the first input DMAs land, so real matmuls start
            # fully ramped (2.4 GHz) instead of paying the 3us ramp
            wu_ps = ps_t.tile([128, 8, 128, 2], FP8, tag="tp")
            for _ in range(PE_WARMUP):
                nc.tensor.transpose(wu_ps[:, 0, :, 0], ident[:], ident[:])

            masks = constp.tile([128, 256], F32, tag="masks")

            # Resident inputs, DMA'd in slot-consumption order (per ORDER):
            # each slot first needs its Q^T block, V^T chunks and V rows up
            # to its extent; the mask rides along after the first slot's data.
            qts = {}   # (d8, j) -> [128, 128] AP
            vts = {}   # (d8, kc) -> [128, 512]
            qt_waves_done = set()
            vh_pair = {}  # g -> [128, 2, D] AP (rows g*256 + i*128 + p)
            vl_pair = {}

            def emit_dma_waves(j, first_rep):
                k = ORDER.index(j)
                # qt waves go out one iteration before their first slot so
                # the wave is resident when that slot's mm1 issues
                for wi, (k0, kn) in enumerate(QT_WAVES):
                    if wi in qt_waves_done or not (k0 - 1 <= k < k0 + kn):
                        continue
                    qt_waves_done.add(wi)
                    t_ = qtp.tile([128, 8, kn * 128], F16, tag=f"qtw{wi}")
                    src = qt_d[:, k0 * 128 : (k0 + kn) * 128].rearrange(
                        "(a p) q -> p a q", p=128
                    )
                    if wi == 0:
                        # d8-halves so the opening matmuls (which consume
                        # d8 in order) start after half the wave
                        nc.sync.dma_start(t_[:, 0:4, :], src[:, 0:4, :])
                        nc.sync.dma_start(t_[:, 4:8, :], src[:, 4:8, :])
                    else:
                        nc.sync.dma_start(t_[:], src)
                    for d8 in range(8):
                        for kk in range(k0, k0 + kn):
                            qts[(d8, ORDER[kk])] = t_[
                                :, d8, (kk - k0) * 128 : (kk - k0 + 1) * 128
                            ]
                for kc in range((EXT[j] + 511) // 512):
                    if (0, kc) in vts:
                        continue
                    t_ = vtp.tile([128, 8, 512], F16, tag=f"vtw{kc}")
                    if kc == 0:
                        # split the first chunk (and its d8-halves) so the
                        # opening matmuls start as soon as possible
                        for hh in (0, 256):
                            src = vt_d[:, hh : hh + 256].rearrange(
                                "(a p) k -> p a k", p=128
                            )
                            if hh == 0:
                                nc.sync.dma_start(t_[:, 0:4, 0:256], src[:, 0:4, :])
                                nc.sync.dma_start(t_[:, 4:8, 0:256], src[:, 4:8, :])
                            else:
                                nc.sync.dma_start(t_[:, :, hh : hh + 256], src)
                    else:
                        nc.sync.dma_start(
                            t_[:],
                            vt_d[:, kc * 512 : (kc + 1) * 512].rearrange(
                                "(a p) k -> p a k", p=128
                            ),
                        )
                    for d8 in range(8):
                        vts[(d8, kc)] = t_[:, d8, :]
                if first_rep and j == ORDER[0]:
                    # needed only after the first mm1 group — keep this off
                    # the head of the DMA queue
                    nc.sync.dma_start(masks[:], mask_d[:, :])

            def emit_front_mm1(j):
                """mm1 for slot j. Returns (j, s_tiles, kws)."""
                E = EXT[j]
                # 512-column score windows (one PSUM bank each, 4 rotating)
                kws = [(c0, min(512, E - c0)) for c0 in range(0, E, 512)]

                # sub-pass width: the opening slot streams 256-wide so its
                # first matmuls only wait on the first V^T half-chunk DMA
                sw_ = 256 if j == ORDER[0] else 512

                s_tiles = []
                for c0, kwd in kws:
                    s_ = ps_s.tile([128, 512], F32, tag="sw")
                    for g0 in range(0, kwd, sw_):
                        gw = min(sw_, kwd - g0)
                        a0 = c0 + g0  # absolute column
                        for d8 in range(8):
                            nc.tensor.matmul(
                                s_[:, g0 : g0 + gw],
                                qts[(d8, j)],
                                vts[(d8, a0 // 512)][
                                    :, a0 % 512 : a0 % 512 + gw
                                ],
                                start=(d8 == 0 and g0 == 0),
                                stop=(d8 == 7 and g0 + gw == kwd),
                            )
                    s_tiles.append(s_)
                return (j, s_tiles, kws)

            def emit_front_stats(j, s_tiles, kws):
                """softmax stats + exp for slot j. Returns (j, w_sb, rinv)."""
                E = EXT[j]
                # additive causal mask on the last 256 columns (DVE)
                lk = len(kws) - 1
                lkd = kws[lk][1]
                nc.vector.tensor_tensor(
                    s_tiles[lk][:, lkd - 256 : lkd],
                    s_tiles[lk][:, lkd - 256 : lkd],
                    masks[:],
                    op=mybir.AluOpType.add,
                )

                # negated row max over the strip
                nmax = None
                for ki, (c0, kwd) in enumerate(kws):
                    nm = statp.tile([128, 1], F32, tag="nm")
                    nc.vector.reduce_max(
                        nm[:], s_tiles[ki][:, :kwd], axis=mybir.AxisListType.X,
                        negate=True,
                    )
                    if nmax is None:
                        nmax = nm
                    else:
                        nm2 = statp.tile([128, 1], F32, tag="nmc")
                        nc.vector.tensor_tensor(
                            nm2[:], nmax[:], nm[:], op=mybir.AluOpType.min
                        )
                        nmax = nm2

                # exp (fused bias) -> fp8 W in SBUF; row-sum of the QUANTIZED
                # weights via DVE so normalization cancels fp8 rounding
                w_sb = wp.tile([128, E], FP8, tag="w")
                rsum = None
                for ki, (c0, kwd) in enumerate(kws):
                    nc.scalar.activation(
                        w_sb[:, c0 : c0 + kwd],
                        s_tiles[ki][:, :kwd],
                        mybir.ActivationFunctionType.Exp,
                        bias=nmax[:],
                    )
                    rs = statp.tile([128, 1], F32, tag="rs")
                    nc.vector.tensor_reduce(
                        rs[:], w_sb[:, c0 : c0 + kwd],
                        axis=mybir.AxisListType.X, op=mybir.AluOpType.add,
                    )
                    if rsum is None:
                        rsum = rs
                    else:
                        rs2 = statp.tile([128, 1], F32, tag="rsc")
                        nc.vector.tensor_add(rs2[:], rsum[:], rs[:])
                        rsum = rs2
                rinv = statp.tile([128, 1], F32, tag="rinv")
                nc.vector.reciprocal(rinv[:], rsum[:])
                return (j, w_sb, rinv)

            def emit_vn_waves(j):
                # one coalesced DMA per 512-row wave carrying BOTH the fp8
                # hi and lo halves, so a pair's operands arrive atomically
                for q_ in range((EXT[j] // 128 + 3) // 4):
                    if q_ * 2 in vh_pair:
                        continue
                    t_ = vnp.tile([128, 2, 4, D], FP8, tag=f"vw{q_}")
                    for ti in range(2):
                        nc.sync.dma_start(
                            t_[:, ti],
                            vhl_d[ti, q_ * 512 : (q_ + 1) * 512, :].rearrange(
                                "(a p) d -> p a d", p=128
                            ),
                        )
                    for li, pair in ((0, vh_pair), (1, vl_pair)):
                        pair[q_ * 2] = t_[:, li, 0:2, :]
                        pair[q_ * 2 + 1] = t_[:, li, 2:4, :]

            def emit_back_tr(state, repack_dve=False):
                """transpose W blocks (PE) + repack to SBUF (ACT/DVE)."""
                j, w_sb, rinv = state
                nblk = EXT[j] // 128
                wt_tiles = []
                for gi, g0 in enumerate(range(0, nblk, 8)):
                    gn = min(8, nblk - g0)
                    # fp8 transpose needs output element step 2
                    t_ps = ps_t.tile([128, 8, 128, 2], FP8, tag="tp")
                    for bi in range(gn):
                        blk = g0 + bi
                        nc.tensor.transpose(
                            t_ps[:, bi, :, 0],
                            w_sb[:, blk * 128 : (blk + 1) * 128],
                            ident[:],
                        )
                    wt_sb = wtp.tile([128, 1024], FP8, tag="wt")
                    # repack split across ScalarE/DVE: on ACT the per-iter
                    # queue order [repack(j-2), exp(j), out-copy(j-2)]
                    # matches dependency completion order; second groups and
                    # tail repacks go to DVE to unload the ACT queue
                    if repack_dve or gi == 1:
                        nc.vector.tensor_copy(
                            wt_sb[:, : gn * 128], t_ps[:, :gn, :, 0]
                        )
                    else:
                        nc.scalar.activation(
                            wt_sb[:, : gn * 128],
                            t_ps[:, :gn, :, 0],
                            mybir.ActivationFunctionType.Copy,
                        )
                    wt_tiles.append(wt_sb)
                return wt_tiles

            def emit_back_mm2(state, wt_tiles, use_sw_psum=False,
                              chunk_major=False):
                """mm2 (fp8 DoubleRow): O[q, d] = W^T.T @ (V_hi + V_lo)."""
                j, w_sb, rinv = state
                npair = EXT[j] // 256
                if use_sw_psum:
                    # tail only: borrow free score windows so the last mm2s
                    # do not wait on the previous slot's out-copies
                    o_w0 = ps_s.tile([128, 512], F32, tag="sw")
                    o_w1 = ps_s.tile([128, 512], F32, tag="sw")
                    o_chunks = [o_w0[:], o_w1[:]]
                else:
                    o_c0 = ps_o.tile([128, 512], F32, tag="oc")
                    o_c1 = ps_o.tile([128, 512], F32, tag="oc")
                    o_chunks = [o_c0[:], o_c1[:]]
                # hi/lo interleaved per pair: consumption matches the
                # arrival order of the V hi/lo waves exactly. chunk_major
                # completes chunk 0's accumulation first so its out-copy
                # and DMA run in the shadow of chunk 1's matmuls (used for
                # the very last mm2, whose store is the critical tail)
                if chunk_major:
                    for ci, dd in enumerate((0, 512)):
                        for g in range(npair):
                            wt_ap = wt_tiles[g // 4][
                                :, (g % 4) * 256 : (g % 4 + 1) * 256
                            ].rearrange("p (i m) -> p i m", i=2)
                            for half, vpair in ((0, vh_pair), (1, vl_pair)):
                                nc.tensor.matmul(
                                    o_chunks[ci],
                                    wt_ap,
                                    vpair[g][:, :, dd : dd + 512],
                                    start=(half == 0 and g == 0),
                                    stop=(half == 1 and g == npair - 1),
                                    perf_mode=mybir.MatmulPerfMode.DoubleRow,
                                )
                    return o_chunks
                for g in range(npair):
                    wt_ap = wt_tiles[g // 4][
                        :, (g % 4) * 256 : (g % 4 + 1) * 256
                    ].rearrange("p (i m) -> p i m", i=2)
                    for half, vpair in ((0, vh_pair), (1, vl_pair)):
                        for ci, dd in enumerate((0, 512)):
                            nc.tensor.matmul(
                                o_chunks[ci],
                                wt_ap,
                                vpair[g][:, :, dd : dd + 512],
                                start=(half == 0 and g == 0),
                                stop=(half == 1 and g == npair - 1),
                                perf_mode=mybir.MatmulPerfMode.DoubleRow,
                            )
                return o_chunks

            def emit_back_store(state, o_chunks, whole=None, dve_in1=None,
                                split_dve=None):
                """normalize (fused 1/rowsum scale) and DMA out."""
                j, w_sb, rinv = state
                o_sb = op.tile([128, 1024], BF16, tag="o")
                if whole is not None and dve_in1 is not None:
                    # final slot: per-chunk copies (DVE, idle at the tail)
                    # each followed by its own DMA so the copy->DMA chains
                    # pipeline; in1 is an initialized SBUF tile, bypassed
                    for ci, dd in enumerate((0, 512)):
                        nc.vector.scalar_tensor_tensor(
                            o_sb[:, dd : dd + 512],
                            whole[:, dd : dd + 512],
                            rinv[:],
                            dve_in1[:, dd : dd + 512],
                            op0=mybir.AluOpType.mult,
                            op1=mybir.AluOpType.bypass,
                        )
                        nc.scalar.dma_start(
                            o_d[j * 128 : (j + 1) * 128, dd : dd + 512],
                            o_sb[:, dd : dd + 512],
                        )
                    return o_sb
                if whole is not None:
                    # single 1024-wide copy across both banks of one window
                    nc.scalar.activation(
                        o_sb[:],
                        whole,
                        mybir.ActivationFunctionType.Copy,
                        scale=rinv[:],
                    )
                elif split_dve is not None:
                    # pipelined final store: chunk 0 (stopped mid-mm2 in
                    # chunk_major order) is copied on ACT and shipped while
                    # chunk 1 still accumulates; chunk 1 then copies on DVE
                    # in parallel with chunk 0's DMA descriptor generation
                    nc.scalar.activation(
                        o_sb[:, 0:512],
                        o_chunks[0],
                        mybir.ActivationFunctionType.Copy,
                        scale=rinv[:],
                    )
                    nc.scalar.dma_start(
                        o_d[j * 128 : (j + 1) * 128, 0:512], o_sb[:, 0:512]
                    )
                    nc.vector.scalar_tensor_tensor(
                        o_sb[:, 512:1024], o_chunks[1], rinv[:], split_dve,
                        op0=mybir.AluOpType.mult, op1=mybir.AluOpType.bypass,
                    )
                    nc.scalar.dma_start(
                        o_d[j * 128 : (j + 1) * 128, 512:1024],
                        o_sb[:, 512:1024],
                    )
                    return o_sb
                else:
                    for ci, dd in enumerate((0, 512)):
                        nc.scalar.activation(
                            o_sb[:, dd : dd + 512],
                            o_chunks[ci],
                            mybir.ActivationFunctionType.Copy,
                            scale=rinv[:],
                        )
                # out DMA from the ACT queue: lands right after its copies
                # and keeps the SP queue free for the input stream
                nc.scalar.dma_start(o_d[j * 128 : (j + 1) * 128, :], o_sb[:])
                return o_sb

            # software pipeline, interleaved so the PE never waits on the
            # softmax handoff: tr(j-2) -> mm1(j) -> mm2(j-2), with slot j's
            # stats emitted before slot j-2's out-copies on the ACT queue
            # and after its repacks on the DVE queue.
            # vh/vl waves lag one slot so they never delay the V^T stream.
            pending = []
            last = len(ORDER) - 1
            for idx, j in enumerate(ORDER):
                emit_dma_waves(j, True)
                if 2 <= idx <= 4:
                    # V hi/lo waves trail this iteration's V^T chunk by half
                    # an iteration: the chunk is consumed mid-mm1, the wave
                    # only by the mm2 that follows it
                    emit_vn_waves(ORDER[idx - 2])
                if idx == last:
                    # final iteration, fully interleaved. stA = the back
                    # whose exp finished an iteration ago: its transposes
                    # and repack (DVE, first in queue) go out first so its
                    # mm2 fills the PE while the last slot's softmax runs;
                    # the big mm2(j-2) then shadows the small backs'
                    # copy->DMA chains; both small mm2s borrow freed score
                    # windows and the last store runs on DVE
                    st7 = pending.pop(0)
                    stA = pending.pop(0)
                    wtA = emit_back_tr(stA, repack_dve=True)
                    wt7 = emit_back_tr(st7)
                    fr = emit_front_mm1(j)
                    ocA = emit_back_mm2(stA, wtA, use_sw_psum=True)
                    stB = emit_front_stats(*fr)
                    emit_back_store(stA, ocA)
                    wtB = emit_back_tr(stB, repack_dve=True)
                    ocB = emit_back_mm2(stB, wtB, use_sw_psum=True)
                    emit_back_store(stB, ocB)
                    oc7 = emit_back_mm2(st7, wt7, chunk_major=True)
                    emit_back_store(st7, oc7, split_dve=wt7[0][:, 0:512])
                    continue
                st = wt = oc = None
                if len(pending) >= 2:
                    st = pending.pop(0)
                    wt = emit_back_tr(st)
                fr = emit_front_mm1(j)
                if st is not None:
                    oc = emit_back_mm2(st, wt)
                if st is not None and idx == last - 1:
                    # near the tail the out-copies beat the exp onto the ACT
                    # queue: they unblock the next mm2's PSUM rotation, and
                    # the exp they delay gates only short mm1s
                    emit_back_store(st, oc)
                    pending.append(emit_front_stats(*fr))
                else:
                    pending.append(emit_front_stats(*fr))
                    if st is not None:
                        emit_back_store(st, oc)
                if idx == 3:
                    # DMA queue is past the V^T stream: flush every
                    # remaining V hi/lo wave now so late mm2s never wait
                    emit_vn_waves(7)

    nc.finalize()
    return nc


_NC_CACHE = None


def _get_program():
    global _NC_CACHE
    if _NC_CACHE is None:
        _NC_CACHE = _build_program()
    return _NC_CACHE


def stage_inputs(query, value):
    """Build the 8 per-core input maps from the full query/value arrays."""
    query = np.asarray(query, dtype=np.float32)
    value = np.asarray(value, dtype=np.float32)

    vhl_b = []
    vt_b = []
    for b in range(B):
        vh = value[b].astype(E4)
        vl = (value[b] - vh.astype(np.float32)).astype(E4)
        vhl_b.append(np.ascontiguousarray(np.stack([vh, vl])))  # [2, T, D]
        vt_b.append(np.ascontiguousarray(value[b].T).astype(np.float16))

    in_maps = []
    for c in range(NCORES):
        b = c % 4
        tiles = _tiles_for_core(c)

        # qt columns grouped by processing ORDER
        q_rows = np.concatenate(
            [query[b, tiles[j] * 128 : (tiles[j] + 1) * 128, :] for j in ORDER],
            axis=0,
        )  # [1024, D]
        qt = np.ascontiguousarray(q_rows.T).astype(np.float16)  # [D, 1024]

        # same additive mask pattern for every slot of this core:
        # odd tiles (c<4): kill cols k where k > 128 + r of the last 256;
        # even tiles (c>=4): kill k > r (incl. the fully-padded last 128)
        r = np.arange(128)[:, None]
        k = np.arange(256)[None, :]
        mask = np.where(k > ((128 + r) if c < 4 else r), -NEG_INF, 0.0).astype(
            np.float32
        )

        in_maps.append(
            {"qt": qt, "vt": vt_b[b], "vhl": vhl_b[b], "mask": mask}
        )
    return in_maps


def kernel(query, value):
    nc = _get_program()
    in_maps = stage_inputs(query, value)
    res = run_bass_kernel_spmd(nc, in_maps, core_ids=list(range(NCORES)))

    out = np.empty((B, T, D), dtype=np.float32)
    for c in range(NCORES):
        o = np.asarray(res.results[c]["o"]).astype(np.float32)  # [1024, D]
        b = c % 4
        for j, t in enumerate(_tiles_for_core(c)):
            out[b, t * 128 : (t + 1) * 128, :] = o[j * 128 : (j + 1) * 128, :]
    return out


# revision 101
# speedup vs baseline: 1.0028x; 1.0028x over previous
"""Causal dense attention (key=value) on 8 TRN2 NeuronCores.

Reference semantics (B=4, T=2048, D=1024, fp32):
    scores  = Q @ V^T                      [B, T, T]
    scores -= 1e9 * (~tril)                causal mask
    W       = softmax(scores, axis=-1)
    out     = W @ V                        [B, T, D]

Sharding: 2 cores per batch. Each batch's 16 causal q-tiles (128 rows
each, kv extent 128*(t+1)) are split odd/even so both cores get the
same padded kv-extent schedule EXT = [256, 512, ..., 2048] (ascending),
making the Bass program identical across all 8 cores (pure SPMD).
Padding columns are killed by the additive causal mask, which is the
same [128, 256] pattern for every slot of a given core.

Host stages per core: Q^T (d-major, columns grouped by processing
order) and V^T (d-major) in fp16 (full TensorE rate at half the HBM
bytes of f32r; scores keep ~10 mantissa bits), V split into fp8e4
hi/lo halves (V = hi + lo at ~bf16 effective precision), and one
additive causal mask tile.

Device pipeline per slot (software-pipelined, lag 2, emitted as
tr(j-2) -> mm1(j) -> mm2(j-2) so the PE never waits on the softmax
handoff):
  mm1  S = Q^T.T @ V^T into PSUM kilo-windows (fp16); DVE adds the
       causal mask to the last 256 columns;
  stats row-max (DVE reduce, negated) -> exp with fused bias to fp8e4
       W in SBUF (ScalarE); DVE row-sum over the quantized W so the
       softmax normalizer matches the fp8 weights exactly;
  mm2  PE-transpose of W blocks (fp8, stride-2 PSUM out), repack on
       ScalarE, then DoubleRow fp8 matmuls W^T.T @ V_hi + W^T.T @ V_lo
       (two k-tiles per instruction, 2x rate) into PSUM, and a ScalarE
       copy fused with the 1/rowsum scale emitting bf16, then DMA out
       from the ACT queue (keeps the SP queue free for inputs).
Input DMAs are coalesced (one strided descriptor-set per wave) and
ordered by consumption; the DMA engine runs gapless until all 10.1
MiB of input have landed. The last three backs are hand-interleaved:
the big mm2 runs last in chunk-major order so its out-copy/DMA chains
pipeline under its own accumulation.
"""

import numpy as np

import concourse.bass as bass
import concourse.mybir as mybir
from concourse import bacc, tile
from concourse.bass_utils import run_bass_kernel_spmd
from concourse.masks import make_identity

import ml_dtypes

B, T, D = 4, 2048, 1024
NCORES = 8
NSLOT = 8
EXT = [256 * (j + 1) for j in range(NSLOT)]  # kv extent per slot
# processing order: smallest slot first (fast start); the two
# cheapest remaining backs (slots 2 and 0) drain the pipeline tail
ORDER = [1, 3, 4, 5, 6, 7, 2, 0]
# qt DRAM column-blocks are laid out in ORDER; waves below are
# (start index into ORDER, slot count) — multi-slot waves keep the
# DRAM descriptor runs >= 512B
QT_WAVES = [(0, 2), (2, 2), (4, 2), (6, 2)]
NEG_INF = 1e9
PE_WARMUP = 8  # dependency-free PE transposes at program start

F32 = mybir.dt.float32
F16 = mybir.dt.float16
BF16 = mybir.dt.bfloat16
FP8 = mybir.dt.float8e4
E4 = ml_dtypes.float8_e4m3


def _tiles_for_core(c):
    """q-tile index (within the batch) for each slot, for core c."""
    if c < 4:
        return [2 * j + 1 for j in range(NSLOT)]  # extents exactly EXT
    return [2 * j for j in range(NSLOT)]  # extents EXT - 128 (padded)


def _build_program():
    nc = bacc.Bacc("TRN2", target_bir_lowering=False)

    qt_d = nc.dram_tensor("qt", [D, NSLOT * 128], F16, kind="ExternalInput")
    vt_d = nc.dram_tensor("vt", [D, T], F16, kind="ExternalInput")
    vhl_d = nc.dram_tensor("vhl", [2, T, D], FP8, kind="ExternalInput")
    mask_d = nc.dram_tensor("mask", [128, 256], F32, kind="ExternalInput")
    o_d = nc.dram_tensor("o", [NSLOT * 128, D], BF16, kind="ExternalOutput")

    with tile.TileContext(nc) as tc:
        with (
            tc.tile_pool(name="const", bufs=1) as constp,
            tc.tile_pool(name="qt", bufs=1) as qtp,
            tc.tile_pool(name="vt", bufs=1) as vtp,
            tc.tile_pool(name="vn", bufs=1) as vnp,
            tc.tile_pool(name="w", bufs=3) as wp,
            tc.tile_pool(name="wt", bufs=8) as wtp,
            tc.tile_pool(name="osb", bufs=4) as op,
            tc.tile_pool(name="stats", bufs=24) as statp,
            tc.tile_pool(name="ps_s", bufs=4, space="PSUM") as ps_s,
            tc.tile_pool(name="ps_t", bufs=2, space="PSUM") as ps_t,
            tc.tile_pool(name="ps_o", bufs=2, space="PSUM") as ps_o,
        ):
            ident = constp.tile([128, 128], FP8, tag="ident")
            make_identity(nc, ident[:])

            # ACT exp-table warm-up: load exp_and_others during initial DMAs
            warm = statp.tile([128, 1], F32, tag="warm")
            nc.gpsimd.memset(warm[:], 0.0)
            nc.scalar.activation(warm[:], warm[:], mybir.ActivationFunctionType.Exp)

            # PE p-state warm-up: dependency-free transposes keep the PE
            # busy while the first input DMAs land, so real matmuls start
            # fully ramped (2.4 GHz) instead of paying the 3us ramp
            wu_ps = ps_t.tile([128, 8, 128, 2], FP8, tag="tp")
            for _ in range(PE_WARMUP):
                nc.tensor.transpose(wu_ps[:, 0, :, 0], ident[:], ident[:])

            masks = constp.tile([128, 256], F32, tag="masks")

            # Resident inputs, DMA'd in slot-consumption order (per ORDER):
            # each slot first needs its Q^T block, V^T chunks and V rows up
            # to its extent; the mask rides along after the first slot's data.
            qts = {}   # (d8, j) -> [128, 128] AP
            vts = {}   # (d8, kc) -> [128, 512]
            qt_waves_done = set()
            vh_pair = {}  # g -> [128, 2, D] AP (rows g*256 + i*128 + p)
            vl_pair = {}

            def emit_dma_waves(j, first_rep):
                k = ORDER.index(j)
                # qt waves go out one iteration before their first slot so
                # the wave is resident when that slot's mm1 issues
                for wi, (k0, kn) in enumerate(QT_WAVES):
                    if wi in qt_waves_done or not (k0 - 1 <= k < k0 + kn):
                        continue
                    qt_waves_done.add(wi)
                    t_ = qtp.tile([128, 8, kn * 128], F16, tag=f"qtw{wi}")
                    src = qt_d[:, k0 * 128 : (k0 + kn) * 128].rearrange(
                        "(a p) q -> p a q", p=128
                    )
                    if wi == 0:
                        # d8-halves so the opening matmuls (which consume
                        # d8 in order) start after half the wave
                        nc.sync.dma_start(t_[:, 0:4, :], src[:, 0:4, :])
                        nc.sync.dma_start(t_[:, 4:8, :], src[:, 4:8, :])
                    else:
                        nc.sync.dma_start(t_[:], src)
                    for d8 in range(8):
                        for kk in range(k0, k0 + kn):
                            qts[(d8, ORDER[kk])] = t_[
                                :, d8, (kk - k0) * 128 : (kk - k0 + 1) * 128
                            ]
                for kc in range((EXT[j] + 511) // 512):
                    if (0, kc) in vts:
                        continue
                    t_ = vtp.tile([128, 8, 512], F16, tag=f"vtw{kc}")
                    if kc == 0:
                        # split the first chunk (and its d8-halves) so the
                        # opening matmuls start as soon as possible
                        for hh in (0, 256):
                            src = vt_d[:, hh : hh + 256].rearrange(
                                "(a p) k -> p a k", p=128
                            )
                            if hh == 0:
                                nc.sync.dma_start(t_[:, 0:4, 0:256], src[:, 0:4, :])
                                nc.sync.dma_start(t_[:, 4:8, 0:256], src[:, 4:8, :])
                            else:
                                nc.sync.dma_start(t_[:, :, hh : hh + 256], src)
                    else:
                        nc.sync.dma_start(
                            t_[:],
                            vt_d[:, kc * 512 : (kc + 1) * 512].rearrange(
                                "(a p) k -> p a k", p=128
                            ),
                        )
                    for d8 in range(8):
                        vts[(d8, kc)] = t_[:, d8, :]
                if first_rep and j == ORDER[0]:
                    # needed only after the first mm1 group — keep this off
                    # the head of the DMA queue
                    nc.sync.dma_start(masks[:], mask_d[:, :])

            def emit_front_mm1(j):
                """mm1 for slot j. Returns (j, s_tiles, kws)."""
                E = EXT[j]
                # 512-column score windows (one PSUM bank each, 4 rotating)
                kws = [(c0, min(512, E - c0)) for c0 in range(0, E, 512)]

                # sub-pass width: the opening slot streams 256-wide so its
                # first matmuls only wait on the first V^T half-chunk DMA
                sw_ = 256 if j == ORDER[0] else 512

                s_tiles = []
                for c0, kwd in kws:
                    s_ = ps_s.tile([128, 512], F32, tag="sw")
                    for g0 in range(0, kwd, sw_):
                        gw = min(sw_, kwd - g0)
                        a0 = c0 + g0  # absolute column
                        for d8 in range(8):
                            nc.tensor.matmul(
                                s_[:, g0 : g0 + gw],
                                qts[(d8, j)],
                                vts[(d8, a0 // 512)][
                                    :, a0 % 512 : a0 % 512 + gw
                                ],
                                start=(d8 == 0 and g0 == 0),
                                stop=(d8 == 7 and g0 + gw == kwd),
                            )
                    s_tiles.append(s_)
                return (j, s_tiles, kws)

            def emit_front_stats(j, s_tiles, kws):
                """softmax stats + exp for slot j. Returns (j, w_sb, rinv)."""
                E = EXT[j]
                # additive causal mask on the last 256 columns (DVE)
                lk = len(kws) - 1
                lkd = kws[lk][1]
                nc.vector.tensor_tensor(
                    s_tiles[lk][:, lkd - 256 : lkd],
                    s_tiles[lk][:, lkd - 256 : lkd],
                    masks[:],
                    op=mybir.AluOpType.add,
                )

                # negated row max over the strip
                nmax = None
                for ki, (c0, kwd) in enumerate(kws):
                    nm = statp.tile([128, 1], F32, tag="nm")
                    nc.vector.reduce_max(
                        nm[:], s_tiles[ki][:, :kwd], axis=mybir.AxisListType.X,
                        negate=True,
                    )
                    if nmax is None:
                        nmax = nm
                    else:
                        nm2 = statp.tile([128, 1], F32, tag="nmc")
                        nc.vector.tensor_tensor(
                            nm2[:], nmax[:], nm[:], op=mybir.AluOpType.min
                        )
                        nmax = nm2

                # exp (fused bias) -> fp8 W in SBUF; row-sum of the QUANTIZED
                # weights via DVE so normalization cancels fp8 rounding
                w_sb = wp.tile([128, E], FP8, tag="w")
                rsum = None
                for ki, (c0, kwd) in enumerate(kws):
                    nc.scalar.activation(
                        w_sb[:, c0 : c0 + kwd],
                        s_tiles[ki][:, :kwd],
                        mybir.ActivationFunctionType.Exp,
                        bias=nmax[:],
                    )
                    rs = statp.tile([128, 1], F32, tag="rs")
                    nc.vector.tensor_reduce(
                        rs[:], w_sb[:, c0 : c0 + kwd],
                        axis=mybir.AxisListType.X, op=mybir.AluOpType.add,
                    )
                    if rsum is None:
                        rsum = rs
                    else:
                        rs2 = statp.tile([128, 1], F32, tag="rsc")
                        nc.vector.tensor_add(rs2[:], rsum[:], rs[:])
                        rsum = rs2
                rinv = statp.tile([128, 1], F32, tag="rinv")
                nc.vector.reciprocal(rinv[:], rsum[:])
                return (j, w_sb, rinv)

            def emit_vn_waves(j):
                # one coalesced DMA per 512-row wave carrying BOTH the fp8
                # hi and lo halves, so a pair's operands arrive atomically
                for q_ in range((EXT[j] // 128 + 3) // 4):
                    if q_ * 2 in vh_pair:
                        continue
                    t_ = vnp.tile([128, 2, 4, D], FP8, tag=f"vw{q_}")
                    for ti in range(2):
                        nc.sync.dma_start(
                            t_[:, ti],
                            vhl_d[ti, q_ * 512 : (q_ + 1) * 512, :].rearrange(
                                "(a p) d -> p a d", p=128
                            ),
                        )
                    for li, pair in ((0, vh_pair), (1, vl_pair)):
                        pair[q_ * 2] = t_[:, li, 0:2, :]
                        pair[q_ * 2 + 1] = t_[:, li, 2:4, :]

            def emit_back_tr(state, repack_dve=False):
                """transpose W blocks (PE) + repack to SBUF (ACT/DVE)."""
                j, w_sb, rinv = state
                nblk = EXT[j] // 128
                wt_tiles = []
                for gi, g0 in enumerate(range(0, nblk, 8)):
                    gn = min(8, nblk - g0)
                    # fp8 transpose needs output element step 2
                    t_ps = ps_t.tile([128, 8, 128, 2], FP8, tag="tp")
                    for bi in range(gn):
                        blk = g0 + bi
                        nc.tensor.transpose(
                            t_ps[:, bi, :, 0],
                            w_sb[:, blk * 128 : (blk + 1) * 128],
                            ident[:],
                        )
                    wt_sb = wtp.tile([128, 1024], FP8, tag="wt")
                    # repack split across ScalarE/DVE: on ACT the per-iter
                    # queue order [repack(j-2), exp(j), out-copy(j-2)]
                    # matches dependency completion order; second groups and
                    # tail repacks go to DVE to unload the ACT queue
                    if repack_dve or gi == 1:
                        nc.vector.tensor_copy(
                            wt_sb[:, : gn * 128], t_ps[:, :gn, :, 0]
                        )
                    else:
                        nc.scalar.activation(
                            wt_sb[:, : gn * 128],
                            t_ps[:, :gn, :, 0],
                            mybir.ActivationFunctionType.Copy,
                        )
                    wt_tiles.append(wt_sb)
                return wt_tiles

            def emit_back_mm2(state, wt_tiles, use_sw_psum=False,
                              chunk_major=False):
                """mm2 (fp8 DoubleRow): O[q, d] = W^T.T @ (V_hi + V_lo)."""
                j, w_sb, rinv = state
                npair = EXT[j] // 256
                if use_sw_psum:
                    # tail only: borrow free score windows so the last mm2s
                    # do not wait on the previous slot's out-copies
                    o_w0 = ps_s.tile([128, 512], F32, tag="sw")
                    o_w1 = ps_s.tile([128, 512], F32, tag="sw")
                    o_chunks = [o_w0[:], o_w1[:]]
                else:
                    o_c0 = ps_o.tile([128, 512], F32, tag="oc")
                    o_c1 = ps_o.tile([128, 512], F32, tag="oc")
                    o_chunks = [o_c0[:], o_c1[:]]
                # hi/lo interleaved per pair: consumption matches the
                # arrival order of the V hi/lo waves exactly. chunk_major
                # completes chunk 0's accumulation first so its out-copy
                # and DMA run in the shadow of chunk 1's matmuls (used for
                # the very last mm2, whose store is the critical tail)
                if chunk_major:
                    for ci, dd in enumerate((0, 512)):
                        for g in range(npair):
                            wt_ap = wt_tiles[g // 4][
                                :, (g % 4) * 256 : (g % 4 + 1) * 256
                            ].rearrange("p (i m) -> p i m", i=2)
                            for half, vpair in ((0, vh_pair), (1, vl_pair)):
                                nc.tensor.matmul(
                                    o_chunks[ci],
                                    wt_ap,
                                    vpair[g][:, :, dd : dd + 512],
                                    start=(half == 0 and g == 0),
                                    stop=(half == 1 and g == npair - 1),
                                    perf_mode=mybir.MatmulPerfMode.DoubleRow,
                                )
                    return o_chunks
                for g in range(npair):
                    wt_ap = wt_tiles[g // 4][
                        :, (g % 4) * 256 : (g % 4 + 1) * 256
                    ].rearrange("p (i m) -> p i m", i=2)
                    for half, vpair in ((0, vh_pair), (1, vl_pair)):
                        for ci, dd in enumerate((0, 512)):
                            nc.tensor.matmul(
                                o_chunks[ci],
                                wt_ap,
                                vpair[g][:, :, dd : dd + 512],
                                start=(half == 0 and g == 0),
                                stop=(half == 1 and g == npair - 1),
                                perf_mode=mybir.MatmulPerfMode.DoubleRow,
                            )
                return o_chunks

            def emit_back_store(state, o_chunks, whole=None, dve_in1=None,
                                split_dve=None):
                """normalize (fused 1/rowsum scale) and DMA out."""
                j, w_sb, rinv = state
                o_sb = op.tile([128, 1024], BF16, tag="o")
                if whole is not None and dve_in1 is not None:
                    # final slot: per-chunk copies (DVE, idle at the tail)
                    # each followed by its own DMA so the copy->DMA chains
                    # pipeline; in1 is an initialized SBUF tile, bypassed
                    for ci, dd in enumerate((0, 512)):
                        nc.vector.scalar_tensor_tensor(
                            o_sb[:, dd : dd + 512],
                            whole[:, dd : dd + 512],
                            rinv[:],
                            dve_in1[:, dd : dd + 512],
                            op0=mybir.AluOpType.mult,
                            op1=mybir.AluOpType.bypass,
                        )
                        nc.scalar.dma_start(
                            o_d[j * 128 : (j + 1) * 128, dd : dd + 512],
                            o_sb[:, dd : dd + 512],
                        )
                    return o_sb
                if whole is not None:
                    # single 1024-wide copy across both banks of one window
                    nc.scalar.activation(
                        o_sb[:],
                        whole,
                        mybir.ActivationFunctionType.Copy,
                        scale=rinv[:],
                    )
                elif split_dve is not None:
                    # pipelined final store: chunk 0 (stopped mid-mm2 in
                    # chunk_major order) is copied on ACT and shipped while
                    # chunk 1 still accumulates; chunk 1 then copies on DVE
                    # in parallel with chunk 0's DMA descriptor generation
                    nc.scalar.activation(
                        o_sb[:, 0:512],
                        o_chunks[0],
                        mybir.ActivationFunctionType.Copy,
                        scale=rinv[:],
                    )
                    nc.scalar.dma_start(
                        o_d[j * 128 : (j + 1) * 128, 0:512], o_sb[:, 0:512]
                    )
                    nc.vector.scalar_tensor_tensor(
                        o_sb[:, 512:1024], o_chunks[1], rinv[:], split_dve,
                        op0=mybir.AluOpType.mult, op1=mybir.AluOpType.bypass,
                    )
                    # final chunk ships from the idle SP queue so its
                    # descriptor generation overlaps chunk 0's on ACT
                    nc.sync.dma_start(
                        o_d[j * 128 : (j + 1) * 128, 512:1024],
                        o_sb[:, 512:1024],
                    )
                    return o_sb
                else:
                    for ci, dd in enumerate((0, 512)):
                        nc.scalar.activation(
                            o_sb[:, dd : dd + 512],
                            o_chunks[ci],
                            mybir.ActivationFunctionType.Copy,
                            scale=rinv[:],
                        )
                # out DMA from the ACT queue: lands right after its copies
                # and keeps the SP queue free for the input stream
                nc.scalar.dma_start(o_d[j * 128 : (j + 1) * 128, :], o_sb[:])
                return o_sb

            # software pipeline, interleaved so the PE never waits on the
            # softmax handoff: tr(j-2) -> mm1(j) -> mm2(j-2), with slot j's
            # stats emitted before slot j-2's out-copies on the ACT queue
            # and after its repacks on the DVE queue.
            # vh/vl waves lag one slot so they never delay the V^T stream.
            pending = []
            last = len(ORDER) - 1
            for idx, j in enumerate(ORDER):
                emit_dma_waves(j, True)
                if 2 <= idx <= 4:
                    # V hi/lo waves trail this iteration's V^T chunk by half
                    # an iteration: the chunk is consumed mid-mm1, the wave
                    # only by the mm2 that follows it
                    emit_vn_waves(ORDER[idx - 2])
                if idx == last:
                    # final iteration, fully interleaved. stA = the back
                    # whose exp finished an iteration ago: its transposes
                    # and repack (DVE, first in queue) go out first so its
                    # mm2 fills the PE while the last slot's softmax runs;
                    # the big mm2(j-2) then shadows the small backs'
                    # copy->DMA chains; both small mm2s borrow freed score
                    # windows and the last store runs on DVE
                    st7 = pending.pop(0)
                    stA = pending.pop(0)
                    wtA = emit_back_tr(stA, repack_dve=True)
                    wt7 = emit_back_tr(st7)
                    fr = emit_front_mm1(j)
                    ocA = emit_back_mm2(stA, wtA, use_sw_psum=True)
                    stB = emit_front_stats(*fr)
                    emit_back_store(stA, ocA)
                    wtB = emit_back_tr(stB, repack_dve=True)
                    ocB = emit_back_mm2(stB, wtB, use_sw_psum=True)
                    emit_back_store(stB, ocB)
                    oc7 = emit_back_mm2(st7, wt7, chunk_major=True)
                    emit_back_store(st7, oc7, split_dve=wt7[0][:, 0:512])
                    continue
                st = wt = oc = None
                if len(pending) >= 2:
                    st = pending.pop(0)
                    wt = emit_back_tr(st)
                fr = emit_front_mm1(j)
                if st is not None:
                    oc = emit_back_mm2(st, wt)
                if st is not None and idx == last - 1:
                    # near the tail the out-copies beat the exp onto the ACT
                    # queue: they unblock the next mm2's PSUM rotation, and
                    # the exp they delay gates only short mm1s
                    emit_back_store(st, oc)
                    pending.append(emit_front_stats(*fr))
                else:
                    pending.append(emit_front_stats(*fr))
                    if st is not None:
                        emit_back_store(st, oc)
                if idx == 3:
                    # DMA queue is past the V^T stream: flush every
                    # remaining V hi/lo wave now so late mm2s never wait
                    emit_vn_waves(7)

    nc.finalize()
    return nc


_NC_CACHE = None


def _get_program():
    global _NC_CACHE
    if _NC_CACHE is None:
        _NC_CACHE = _build_program()
    return _NC_CACHE


def stage_inputs(query, value):
    """Build the 8 per-core input maps from the full query/value arrays."""
    query = np.asarray(query, dtype=np.float32)
    value = np.asarray(value, dtype=np.float32)

    vhl_b = []
    vt_b = []
    for b in range(B):
        vh = value[b].astype(E4)
        vl = (value[b] - vh.astype(np.float32)).astype(E4)
        vhl_b.append(np.ascontiguousarray(np.stack([vh, vl])))  # [2, T, D]
        vt_b.append(np.ascontiguousarray(value[b].T).astype(np.float16))

    in_maps = []
    for c in range(NCORES):
        b = c % 4
        tiles = _tiles_for_core(c)

        # qt columns grouped by processing ORDER
        q_rows = np.concatenate(
            [query[b, tiles[j] * 128 : (tiles[j] + 1) * 128, :] for j in ORDER],
            axis=0,
        )  # [1024, D]
        qt = np.ascontiguousarray(q_rows.T).astype(np.float16)  # [D, 1024]

        # same additive mask pattern for every slot of this core:
        # odd tiles (c<4): kill cols k where k > 128 + r of the last 256;
        # even tiles (c>=4): kill k > r (incl. the fully-padded last 128)
        r = np.arange(128)[:, None]
        k = np.arange(256)[None, :]
        mask = np.where(k > ((128 + r) if c < 4 else r), -NEG_INF, 0.0).astype(
            np.float32
        )

        in_maps.append(
            {"qt": qt, "vt": vt_b[b], "vhl": vhl_b[b], "mask": mask}
        )
    return in_maps


def kernel(query, value):
    nc = _get_program()
    in_maps = stage_inputs(query, value)
    res = run_bass_kernel_spmd(nc, in_maps, core_ids=list(range(NCORES)))

    out = np.empty((B, T, D), dtype=np.float32)
    for c in range(NCORES):
        o = np.asarray(res.results[c]["o"]).astype(np.float32)  # [1024, D]
        b = c % 4
        for j, t in enumerate(_tiles_for_core(c)):
            out[b, t * 128 : (t + 1) * 128, :] = o[j * 128 : (j + 1) * 128, :]
    return out
